# revision 57
# baseline (speedup 1.0000x reference)
"""Multi-head attention (B=4, L=2048, D=1024, H=16) on 8 trn2 NeuronCores.

Sharding: core c = 2*b + g handles batch b and head-group g (8 heads = 512
dims).  Each core computes Q/K/V projections for its group, attention for its
8 heads, and a partial output projection ctx_g @ Wo[g*512:(g+1)*512, :].  The
host sums the two group partials per batch.

v19 (per core) — fp8 DoubleRow + flipped-ctx design, 186978 ns cost-model
span @ rel_err 1.22e-2 (vs 246211 bf16 baseline):
  proj     : split-fp8e4m3 DoubleRow chains (256-deep contraction, 0.5
             cy/row).  x pre-split hi+lo on the host; W pre-scaled by 32
             (fp8 subnormal dodge).  Q/K use 2 terms -- (x_hi+x_lo)@W_hi,
             x fully compensated, W single-quantized (~1.1%% on the
             output via scores only); V keeps all 3 compensation terms
             (value path).  wqlo/wklo are never loaded.
  Q/K      : Q stored as on-device hi/lo fp8 pair (DVE: hi=f8(ps),
             lo=f8(ps+bq-hi)); K as single fp8 (bias folded).  One DoubleRow
             matmul per (head, key-tile) computes (Q_hi+Q_lo)^T K exactly via
             the two DR k-tile slots with a stride-0-broadcast K operand.
             The 1/sqrt(dh) scale and the 32*32 weight prescale fold into the
             exp's scale immediate (1/8192); padd folds into the exp bias.
  ctx      : flipped orientation out[q, d] = probs^T V per 128-query slice:
             65 output columns (64 V dims + ones col accumulating Z).  Z
             lands on the query partition, so normalization is one DVE
             scalar_tensor_tensor with a per-partition 1/Z operand.
             Normalized pair tile [128q, 128d] is PE-transposed (identity
             ifmap) back to feature-major for the bf16 output projection;
             partials leave as bf16 and the host sums them in fp32.
  schedule : software pipeline: pair j's score DRs + exps interleave into
             pair (j-1)'s ctx/normalize/transpose drain (ACT exp is the
             ~147us floor, 79%% of span); each score is sandwiched inside
             a projection chain's DR halves (scores use mm2p, chains own
             the mmp ring, so the open accumulation is safe); out-proj is
             deferred two blocks (ctxT bufs=3) so the exp-bound final
             block absorbs it into its exp-WAR gaps; the last pair's
             drain is kt-gated into its own score stream.  Startup x/bias
             DMAs issue from the Pool queue in parallel with SP weight
             loads; mid-kernel x prefetches use the then-idle SP queue.
"""

import sys

if "/opt/trn_rl_repo" not in sys.path:
    sys.path.insert(0, "/opt/trn_rl_repo")

import numpy as np
from ml_dtypes import bfloat16, float8_e4m3

B, L, D, H = 4, 2048, 1024, 16
G = 2                # head-groups == cores per batch
DG = D // G          # 512 dims per group
HG = H // G          # 8 heads per group
DH = D // H          # 64
NCORES = B * G
NT = L // 512        # query 512-blocks
NKT = L // 128       # key 128-tiles
ND = D // 128        # contraction chunks over input dim
NP = ND // 2         # DoubleRow chunk-pairs
NJ = DG // 128       # dcol tiles per group (2 heads each)

WSCALE = 32.0        # host prescale on Wq/Wk/Wv (fp8 subnormal dodge)
EXP_SCALE = 1.0 / (8.0 * WSCALE * WSCALE)   # 1/sqrt(DH) / WSCALE^2

MM_DTYPE = "bfloat16"

_cache = {}


def _bcast_ap(ap, n, axis):
    """Insert a stride-0 dim of size n at free-dim position `axis`."""
    import concourse.bass as bass

    dims = [list(d) for d in ap.ap]
    dims.insert(axis, [0, n])
    return bass.AP(tensor=ap.tensor, offset=ap.offset, ap=dims)


def _build_fast():
    import concourse.bass as bass
    import concourse.tile as tile
    from concourse import bacc, library_config, mybir

    f32 = mybir.dt.float32
    bf16 = mybir.dt.bfloat16
    f8 = mybir.dt.float8e4
    AF = mybir.ActivationFunctionType
    ALU = mybir.AluOpType
    DR = mybir.MatmulPerfMode.DoubleRow

    nc = bacc.Bacc("TRN2")

    xin = {}
    for nm in ("xq", "xk", "xv"):
        for p in ("hi", "lo"):
            xin[nm + p] = nc.dram_tensor(nm + p, [D, L], f8,
                                         kind="ExternalInput")
    win = {}
    # wqlo/wklo are unused since the 2-term QK projection: only V needs
    # its W_lo compensation term
    for nm in ("wqhi", "wkhi", "wvhi", "wvlo"):
        win[nm] = nc.dram_tensor(nm, [D, DG], f8, kind="ExternalInput")
    bq_d = nc.dram_tensor("bq", [128, NJ], f32, kind="ExternalInput")
    bk_d = nc.dram_tensor("bk", [128, NJ], f32, kind="ExternalInput")
    bv_d = nc.dram_tensor("bv", [DG], f32, kind="ExternalInput")
    wo_d = nc.dram_tensor("wo", [DG, D], bf16, kind="ExternalInput")
    pdk_d = nc.dram_tensor("paddk", [128, NKT], f32, kind="ExternalInput")
    id_d = nc.dram_tensor("ident", [128, 128], bf16, kind="ExternalInput")
    # bf16 partials: host sums in fp32; halves the out DMA bytes
    out_d = nc.dram_tensor("out", [L, D], bf16, kind="ExternalOutput")

    with tile.TileContext(nc) as tc, (
            __import__("contextlib").ExitStack()) as ctx:
        ep = ctx.enter_context
        wpool = ep(tc.tile_pool(name="persist", bufs=1))
        qkpool = ep(tc.tile_pool(name="qk", bufs=1))
        vpool = ep(tc.tile_pool(name="vaug", bufs=1))
        xpool = ep(tc.tile_pool(name="xin", bufs=1))
        ppool = ep(tc.tile_pool(name="pexp", bufs=28))
        ncpool = ep(tc.tile_pool(name="nctx", bufs=2))
        ctpool = ep(tc.tile_pool(name="ctxt", bufs=3))
        rzpool = ep(tc.tile_pool(name="rzc", bufs=4))
        opool = ep(tc.tile_pool(name="outsb", bufs=4))
        mmp = ep(tc.tile_pool(name="mm", bufs=2, space="PSUM"))
        mm2p = ep(tc.tile_pool(name="mm2", bufs=2, space="PSUM"))
        ctxp = ep(tc.tile_pool(name="ctxps", bufs=2, space="PSUM"))

        # ---- persistent SBUF ----
        w_sb = {}   # name -> [128, ND, DG] fp8
        qhl = [qkpool.tile([128, 2, L], f8, tag=f"qhl{j}", name="qhl")
               for j in range(NJ)]
        kt8 = [qkpool.tile([128, L], f8, tag=f"kt{j}", name="kt8")
               for j in range(NJ)]
        vaug = [vpool.tile([128, HG * 65], bf16, tag=f"va{k}", name="vaug")
                for k in range(NKT)]
        ones8 = wpool.tile([128, HG, 1], f32, tag="ones8")
        nc.vector.memset(ones8, 1.0)
        # preload the Exp table while ACT is idle, so the first real exp
        # doesn't pay ACT_TABLE_LOAD on the critical path
        exp_warm = wpool.tile([1, 1], f32, tag="expwarm")
        nc.scalar.activation(out=exp_warm, in_=ones8[0:1, 0, 0:1],
                             func=AF.Exp)
        for kt in range(NKT):
            v3 = vaug[kt].rearrange("p (h d) -> p h d", h=HG)
            # ones cols on DVE: keeps ACT's stream exp-only
            nc.vector.tensor_copy(out=v3[:, :, 64:65], in_=ones8)

        bq_sb = bk_sb = bv_sb = wo_sb = pdk_sb = id_sb = None

        def load_w(nm, halves=1):
            w_bg = wpool.tile([128, ND, DG], f8, tag=nm, name=nm)
            step = ND // halves
            for hh in range(halves):
                nc.sync.dma_start(
                    out=w_bg[:, step * hh:step * (hh + 1), :],
                    in_=win[nm][128 * step * hh:128 * step * (hh + 1),
                                :].rearrange("(i p) n -> p i n", p=128))
            w_sb[nm] = w_bg

        def load_x(nm, t, n_split=1, eng=None):
            # startup loads go on the Pool queue (parallel with the SP
            # weight-load sequence); mid-kernel prefetches go on SP (idle
            # there) -- Pool's ~1us SWDGE generation per DMA would queue
            # ahead of the in-order affine_selects and delay diag ctx
            ts = slice(512 * t, 512 * (t + 1))
            out = {}
            for p in ("hi", "lo"):
                xt = xpool.tile([128, ND, 512], f8, tag="xt",
                                name="xt", bufs=12)
                step = ND // n_split
                for hh in range(n_split):
                    (eng or nc.sync).dma_start(
                        out=xt[:, step * hh:step * (hh + 1), :],
                        in_=xin[nm + p][128 * step * hh:128 * step * (hh + 1),
                                        ts].rearrange("(i p) n -> p i n",
                                                      p=128))
                out[p] = xt
            return out

        def setup_bias():
            # Pool queue: keeps these off the critical SP weight-load
            # sequence at startup
            nonlocal bq_sb, bk_sb, pdk_sb
            bq_sb = wpool.tile([128, NJ], f32, tag="bq")
            nc.gpsimd.dma_start(out=bq_sb, in_=bq_d[:, :])
            bk_sb = wpool.tile([128, NJ], f32, tag="bk")
            nc.gpsimd.dma_start(out=bk_sb, in_=bk_d[:, :])
            pdk_sb = wpool.tile([128, NKT], f32, tag="pdk")
            nc.gpsimd.dma_start(out=pdk_sb, in_=pdk_d[:, :])

        def setup_rest():
            nonlocal bv_sb, wo_sb, id_sb
            bv_sb = wpool.tile([128, DG], f32, tag="bv")
            bv_ap = bv_d[:]
            nc.sync.dma_start(out=bv_sb, in_=_bcast_ap(bv_ap, 128, 0))
            id_sb = wpool.tile([128, 128], bf16, tag="ident")
            nc.sync.dma_start(out=id_sb, in_=id_d[:, :])
            wo_bg = wpool.tile([128, NJ, D], bf16, tag="wob", name="wo_bg")
            nc.sync.dma_start(
                out=wo_bg, in_=wo_d[:, :].rearrange("(j p) n -> p j n", p=128))
            wo_sb = [wo_bg[:, j, :] for j in range(NJ)]

        # ---- projection chains (12 DoubleRow matmuls each) ----
        def dr_terms(whi, wlo, xhi, xlo):
            return ((whi, xhi), (whi, xlo), (wlo, xhi))

        def qk_chain_parts(which, j, xs, t):
            """Projection chain as fine sub-closures (DR terms + quantize)
            so fills interleave at ~0.4us granularity.  Q/K use 2 terms:
            (x_hi + x_lo) @ W_hi -- x fully compensated, W single-
            quantized; the ~2.4% W error only perturbs the scores
            (~1.1% on the output), inside budget.  V keeps 3 terms."""
            ts = slice(512 * t, 512 * (t + 1))
            whi, wlo = w_sb[which + "hi"], None
            st = {}
            nterm = 2

            def term(ti, wt_xt):
                def run():
                    if "ps" not in st:
                        st["ps"] = mmp.tile([128, 512], f32, tag="mm", name="ps")
                    wt, xt = wt_xt()
                    for p in range(NP):
                        nc.tensor.matmul(
                            out=st["ps"],
                            lhsT=wt[:, 2 * p:2 * p + 2,
                                    128 * j:128 * (j + 1)],
                            rhs=xt[:, 2 * p:2 * p + 2, :],
                            start=(ti == 0 and p == 0),
                            stop=(ti == nterm - 1 and p == NP - 1),
                            perf_mode=DR, skip_group_check=True)
                return run

            def quant():
                ps = st["ps"]
                if which == "wq":
                    nc.vector.tensor_copy(out=qhl[j][:, 0, ts], in_=ps)
                    nc.vector.scalar_tensor_tensor(
                        out=qhl[j][:, 1, ts], in0=ps,
                        scalar=bq_sb[:, j:j + 1],
                        in1=qhl[j][:, 0, ts], op0=ALU.add, op1=ALU.subtract)
                else:
                    nc.vector.tensor_scalar_add(
                        out=kt8[j][:, ts], in0=ps, scalar1=bk_sb[:, j:j + 1])

            terms = [(lambda: (whi, xs["hi"])), (lambda: (whi, xs["lo"])),
                     (lambda: (wlo, xs["hi"]))][:nterm]
            return [term(i, wx) for i, wx in enumerate(terms)] + [quant]

        def qk_chain_halves(which, j, xs, t):
            """[first DR term, rest + quantize]: a score emission can
            sit between the halves (scores touch only mm2p, so the open
            mmp accumulation is safe)."""
            p = qk_chain_parts(which, j, xs, t)
            return [p[0], lambda: [cl() for cl in p[1:]]]

        def qk_chain(which, j, xs, t):
            for cl in qk_chain_parts(which, j, xs, t):
                cl()

        def v_chain_parts(s, t, xs):
            whi, wlo = w_sb["wvhi"], w_sb["wvlo"]
            st = {}

            def term(ti, wt_xt):
                def run():
                    if "ps" not in st:
                        st["ps"] = mmp.tile([128, 512], f32, tag="mm", name="ps")
                    wt, xt = wt_xt()
                    for p in range(NP):
                        nc.tensor.matmul(
                            out=st["ps"],
                            lhsT=xt[:, 2 * p:2 * p + 2,
                                    128 * s:128 * (s + 1)],
                            rhs=wt[:, 2 * p:2 * p + 2, :],
                            start=(ti == 0 and p == 0),
                            stop=(ti == 2 and p == NP - 1),
                            perf_mode=DR, skip_group_check=True)
                return run

            def vout():
                kt = 4 * t + s
                v3 = vaug[kt].rearrange("p (h d) -> p h d", h=HG)
                nc.vector.scalar_tensor_tensor(
                    out=v3[:, :, 0:64],
                    in0=st["ps"].rearrange("p (h d) -> p h d", h=HG),
                    scalar=1.0 / WSCALE,
                    in1=bv_sb.rearrange("p (h d) -> p h d", h=HG),
                    op0=ALU.mult, op1=ALU.add)

            terms = [(lambda: (whi, xs["hi"])), (lambda: (whi, xs["lo"])),
                     (lambda: (wlo, xs["hi"]))]
            return [term(i, wx) for i, wx in enumerate(terms)] + [vout]

        def v_chain_halves(s, t, xs):
            p = v_chain_parts(s, t, xs)
            return [lambda: (p[0](), p[1]()), lambda: (p[2](), p[3]())]

        def v_chain(s, t, xs):
            for cl in v_chain_parts(s, t, xs):
                cl()

        # ---- attention ----
        def scores_one(j, t, kt):
            """One (pair, key-tile): 2 score DRs + 1 exp (+ diag mask).
            Returns the probs tile [128k, 2, 512q] bf16."""
            r = kt - 4 * t
            q0 = 128 * r if r > 0 else 0
            qs = slice(512 * t + q0, 512 * (t + 1))
            ks = slice(128 * kt, 128 * (kt + 1))
            ps2 = mm2p.tile([128, 1024], f32, tag="mm2")
            ps3 = ps2.rearrange("p (h n) -> p h n", h=2)
            for half in range(2):
                ro = 64 * half
                ka = kt8[j][ro:ro + 64, ks]
                nc.tensor.matmul(
                    out=ps3[:, half, q0:512],
                    lhsT=_bcast_ap(ka, 2, 1),
                    rhs=qhl[j][ro:ro + 64, :, qs],
                    start=True, stop=True, perf_mode=DR,
                    skip_group_check=True)
            pe = ppool.tile([128, 2, 512], bf16, tag="pexp")
            nc.scalar.activation(
                out=pe[:, :, q0:512], in_=ps3[:, :, q0:512],
                func=AF.Exp, bias=pdk_sb[:, kt:kt + 1], scale=EXP_SCALE)
            if r >= 0:
                nc.gpsimd.affine_select(
                    out=pe[:, :, q0:q0 + 128], in_=pe[:, :, q0:q0 + 128],
                    pattern=[[0, 2], [1, 128]],
                    compare_op=ALU.is_ge, fill=0.0,
                    base=0, channel_multiplier=-1)
            return pe

        def drain_closures(j, t, pes, ctxT):
            """Pair j's ctx accumulation + normalize + transpose + copy-out.
            Returns a list of 4 per-slice closure segments, to interleave
            with pair j+1's scores."""
            state = {}
            nct = ncpool.tile([128, 4, 2, 64], bf16, tag="nctx", name="nctx")
            cls = []

            def ctx_mm(s, half, kt):
                def run():
                    key = ("c", s)
                    if key not in state:
                        state[key] = ctxp.tile([128, 512], f32, tag="ctx",
                                               name="ctx_ps")
                    cps = state[key]
                    h = 2 * j + half
                    nc.tensor.matmul(
                        out=cps[:, 256 * half:256 * half + 65],
                        lhsT=pes[kt][:, half, 128 * s:128 * (s + 1)],
                        rhs=vaug[kt][:, 65 * h:65 * (h + 1)],
                        start=(kt == 0), stop=(kt == 4 * t + s),
                        skip_group_check=True)
                return run

            def norm(s):
                def run():
                    cps = state[("c", s)]
                    rzt = rzpool.tile([128, 2], f32, tag="rz")
                    zin = cps[:, 64:65]
                    with nc.allow_low_precision(reason="1/Z"):
                        nc.vector.reciprocal(
                            out=rzt, in_=bass.AP(
                                tensor=zin.tensor, offset=zin.offset,
                                ap=[list(zin.ap[0]), [256, 2]]))
                    c0 = cps[:, 0:64]
                    cin = bass.AP(tensor=c0.tensor, offset=c0.offset,
                                  ap=[list(c0.ap[0]), [256, 2], [1, 64]])
                    rb = rzt[:, 0:2]
                    rbc = bass.AP(tensor=rb.tensor, offset=rb.offset,
                                  ap=[list(rb.ap[0]), [1, 2], [0, 64]])
                    nc.vector.scalar_tensor_tensor(
                        out=nct[:, s, :, :], in0=cin, scalar=1.0,
                        in1=rbc, op0=ALU.mult, op1=ALU.mult)
                return run

            def pet(s):
                def run():
                    pt = mmp.tile([128, 512], f32, tag="mm")
                    ptb = pt[:, :].bitcast(bf16)
                    nc.tensor.matmul(
                        out=ptb[:, 0:128], lhsT=nct[:, s, :, :], rhs=id_sb,
                        start=True, stop=True, is_transpose=True,
                        skip_group_check=True)
                    nc.vector.tensor_copy(
                        out=ctxT[j][:, 128 * s:128 * (s + 1)],
                        in_=ptb[:, 0:128])
                return run

            segs = []
            for s in range(4):
                seg = []
                for half in range(2):
                    for kt in range(4 * t + s + 1):
                        seg.append(ctx_mm(s, half, kt))
                seg.append(norm(s))
                seg.append(pet(s))
                segs.append(seg)
            return segs

        def out_proj_closures(t, ctxT):
            def one(s, e):
                def run():
                    es = slice(512 * e, 512 * (e + 1))
                    ps = mmp.tile([128, 512], f32, tag="mm")
                    for jt in range(NJ):
                        nc.tensor.matmul(
                            out=ps,
                            lhsT=ctxT[jt][:, 128 * s:128 * (s + 1)],
                            rhs=wo_sb[jt][:, es],
                            start=(jt == 0), stop=(jt == NJ - 1),
                            skip_group_check=True)
                    ob = opool.tile([128, 512], bf16, tag="ob")
                    nc.vector.tensor_copy(out=ob, in_=ps)
                    r0 = 512 * t + 128 * s
                    nc.sync.dma_start(out=out_d[r0:r0 + 128, es], in_=ob)
                return run
            return [one(s, e) for s in range(4) for e in range(2)]

        # ---- emission schedule ----
        # startup: critical-path DMAs (wq/xq/wk/xk), then only pair 0's
        # Q/K chains; everything else flows in as fillers.
        load_w("wqhi", halves=2)
        xs0 = {"xq": load_x("xq", 0, n_split=2, eng=nc.gpsimd)}
        load_w("wkhi")
        xs0["xk"] = load_x("xk", 0, eng=nc.gpsimd)
        setup_bias()
        qk_chain("wq", 0, xs0["xq"], 0)
        qk_chain("wk", 0, xs0["xk"], 0)
        load_w("wvhi")
        load_w("wvlo")
        xs0["xv"] = load_x("xv", 0, eng=nc.gpsimd)
        # after the startup x-DMA issues so Pool's DMA generation isn't
        # delayed; still ahead of every affine_select in Pool's queue
        nc.gpsimd.load_library(library_config.attn)
        setup_rest()

        for j in range(1, NJ):
            qk_chain("wq", j, xs0["xq"], 0)
        for j in range(1, NJ):
            qk_chain("wk", j, xs0["xk"], 0)
        for s in range(4):
            v_chain(s, 0, xs0["xv"])
        startup_fills = []

        pend_segs = []        # previous pair's drain closure segments
        op_q = []             # deferred out-proj closures: (block, items)
        ctxT_cur = None

        for t in range(NT):
            # issue x(t+1) DMAs + build fillers for this block
            fills = list(startup_fills)
            startup_fills = []
            if t + 1 < NT:
                xs = {nm: load_x(nm, t + 1) for nm in ("xq", "xk", "xv")}
                for j in range(NJ):
                    fills.append(qk_chain_halves("wq", j, xs["xq"], t + 1))
                    fills.append(qk_chain_halves("wk", j, xs["xk"], t + 1))
                for s in range(4):
                    fills.append(v_chain_halves(s, t + 1, xs["xv"]))
            # out-proj deferred two blocks: t=3 (ACT-bound, PE 30us slack)
            # absorbs most of it, filling PE's exp-WAR gaps and avoiding
            # the p-state ramp penalty; ctxT pool bufs=3 keeps the
            # deferred tiles alive
            cutoff = t - 2 if t + 1 < NT else t - 1
            while op_q and op_q[0][0] <= cutoff:
                fills.extend([cl] for cl in op_q.pop(0)[1])

            ctxT_new = [ctpool.tile([128, 512], bf16, tag=f"ct{j}",
                                    name="ctxT") for j in range(NJ)]

            for j in range(NJ):
                nkt_t = 4 * t + 4
                last = (t == NT - 1 and j == NJ - 1)
                drain = [cl for seg in pend_segs for cl in seg]
                # budget fillers for this pair slot
                nfill = len(fills) if last else (
                    (len(fills) + NJ - 1 - j) // (NJ - j))
                pes = []
                if last:
                    # gated self-drain: this pair has no successor to hide
                    # its drain under, so fold slices 0/1 (and their
                    # out-proj) into its own score stream
                    own = drain_closures(j, t, pes, ctxT_new)
                    oproj = out_proj_closures(t, ctxT_new)
                    ptr = [0, 0, 0, 0]

                    def emit_own(i):
                        # ctx matmuls gated two exps back; when a slice's
                        # chain completes, flush its norm+transpose+out-proj.
                        # Slices 2/3 reuse the ctxp bank of slices 0/1, so
                        # they may only start once the earlier slice's norm
                        # (the bank's last reader) is emitted.
                        for s in range(4):
                            if s >= 2 and ptr[s - 2] <= len(own[s - 2]):
                                break
                            seg = own[s]
                            nmm = len(seg) - 2      # ctx matmuls in seg
                            per_half = nmm // 2
                            while ptr[s] < nmm:
                                kt = ptr[s] % per_half
                                if kt > i - 2:
                                    break
                                seg[ptr[s]]()
                                ptr[s] += 1
                            if ptr[s] == nmm:
                                seg[nmm]()
                                seg[nmm + 1]()
                                oproj[2 * s]()
                                oproj[2 * s + 1]()
                                ptr[s] = len(seg) + 1   # done marker

                dp = fp = 0
                for i in range(nkt_t):
                    ftarget = (nfill * (i + 1)) // nkt_t
                    # sandwich the score between the halves of one fill
                    # item (chains touch only the mmp ring; scores touch
                    # mm2p, so the open accumulation is safe)
                    item = None
                    if fp < ftarget and fills:
                        item = fills.pop(0)
                        fp += 1
                        item[0]()
                    pes.append(scores_one(j, t, i))
                    if item:
                        for cl in item[1:]:
                            cl()
                    if last:
                        # predecessor drain over the first half; own-drain
                        # (gated) only after it is fully emitted, else the
                        # PSUM ring slot release deadlocks behind us
                        dtarget = min(len(drain),
                                      (len(drain) * 2 * (i + 1)) // nkt_t)
                    else:
                        dtarget = (len(drain) * (i + 1)) // nkt_t
                    while dp < dtarget:
                        drain[dp]()
                        dp += 1
                    while fp < ftarget and fills:
                        for cl in fills.pop(0):
                            cl()
                        fp += 1
                    if last and dp == len(drain):
                        emit_own(i)
                while dp < len(drain):
                    drain[dp]()
                    dp += 1
                if last:
                    # finish any unflushed slices: all ctx/norm/transpose
                    # first (the s3 critical chain must not sit behind
                    # s2's out-proj), out-projs after
                    late = []
                    for s in range(4):
                        seg = own[s]
                        if ptr[s] > len(seg):
                            continue    # fully flushed in emit_own
                        for cl in seg[min(ptr[s], len(seg)):]:
                            cl()
                        late.extend((oproj[2 * s], oproj[2 * s + 1]))
                        ptr[s] = len(seg) + 1
                    for cl in late:
                        cl()
                    pend_segs = []
                    pes_hold = pes    # keep probs tiles referenced
                else:
                    pend_segs = drain_closures(j, t, pes, ctxT_new)
            ctxT_cur = ctxT_new
            if t + 1 < NT:
                op_q.append((t, out_proj_closures(t, ctxT_cur)))

    nc.finalize()
    return nc


# ---------------------------------------------------------------------------
# generic-mask fallback (bf16, mask supplied as data) -- from v6
# ---------------------------------------------------------------------------

def _build_generic(mm_dtype_name):
    import concourse.bass as bass
    import concourse.tile as tile
    from concourse import bacc, library_config, mybir

    f32 = mybir.dt.float32
    AF = mybir.ActivationFunctionType
    ALU = mybir.AluOpType
    mdt = getattr(mybir.dt, mm_dtype_name)

    nc = bacc.Bacc("TRN2")

    xqt = nc.dram_tensor("xqt", [D, L], mdt, kind="ExternalInput")
    xkt = nc.dram_tensor("xkt", [D, L], mdt, kind="ExternalInput")
    xvt = nc.dram_tensor("xvt", [D, L], mdt, kind="ExternalInput")
    wq_d = nc.dram_tensor("wq", [D, DG], mdt, kind="ExternalInput")
    wk_d = nc.dram_tensor("wk", [D, DG], mdt, kind="ExternalInput")
    wv_d = nc.dram_tensor("wv", [D, DG], mdt, kind="ExternalInput")
    bq_d = nc.dram_tensor("bq", [128, NJ], f32, kind="ExternalInput")
    bk_d = nc.dram_tensor("bk", [128, NJ], f32, kind="ExternalInput")
    bv_d = nc.dram_tensor("bv", [DG], f32, kind="ExternalInput")
    wo_d = nc.dram_tensor("wo", [DG, D], mdt, kind="ExternalInput")
    msk_d = nc.dram_tensor("maskt", [L, L], f32, kind="ExternalInput")
    out_d = nc.dram_tensor("out", [L, D], f32, kind="ExternalOutput")

    with tile.TileContext(nc) as tc, (
        __import__("contextlib").ExitStack()) as ctx:
        ep = ctx.enter_context
        wpool = ep(tc.tile_pool(name="persist", bufs=1))
        qkpool = ep(tc.tile_pool(name="qk", bufs=1))
        vpool = ep(tc.tile_pool(name="vaug", bufs=1))
        mmp = ep(tc.tile_pool(name="mm", bufs=2, space="PSUM"))
        mm2p = ep(tc.tile_pool(name="mm2", bufs=2, space="PSUM"))
        ctxp = ep(tc.tile_pool(name="ctxps", bufs=2, space="PSUM"))
        ppool = ep(tc.tile_pool(name="pexp", bufs=4))
        ctpool = ep(tc.tile_pool(name="ctxt", bufs=8))
        rzpool = ep(tc.tile_pool(name="rzc", bufs=2))
        rbpool = ep(tc.tile_pool(name="rbc", bufs=2))
        opool = ep(tc.tile_pool(name="outsb", bufs=4))

        nc.gpsimd.load_library(library_config.attn)

        wq_sb = wk_sb = wv_sb = bq_sb = bk_sb = bv_sb = None

        qt_sb = [qkpool.tile([128, L], mdt, tag=f"qt{j}", name="qt_sb")
                 for j in range(NJ)]
        kt_sb = [qkpool.tile([128, L], mdt, tag=f"kt{j}", name="kt_sb")
                 for j in range(NJ)]
        vaug = [vpool.tile([128, HG * 65], mdt, tag=f"va{k}", name="vaug")
                for k in range(NKT)]
        ones8 = wpool.tile([128, HG, 1], f32, tag="ones8")
        nc.vector.memset(ones8, 1.0)
        for kt in range(NKT):
            v3 = vaug[kt].rearrange("p (h d) -> p h d", h=HG)
            nc.scalar.copy(out=v3[:, :, 64:65], in_=ones8)

        def load_w3(which):
            nonlocal wq_sb, wk_sb, wv_sb, bq_sb, bk_sb, bv_sb
            if which == "q":
                wq_bg = wpool.tile([128, ND, DG], mdt, tag="wqb", name="wq_bg")
                nc.sync.dma_start(
                    out=wq_bg,
                    in_=wq_d[:, :].rearrange("(i p) n -> p i n", p=128))
                wq_sb = [wq_bg[:, i, :] for i in range(ND)]
                bq_sb = wpool.tile([128, NJ], f32, tag="bq")
                nc.sync.dma_start(out=bq_sb, in_=bq_d[:, :])
            elif which == "k":
                wk_bg = wpool.tile([128, ND, DG], mdt, tag="wkb", name="wk_bg")
                nc.sync.dma_start(
                    out=wk_bg,
                    in_=wk_d[:, :].rearrange("(i p) n -> p i n", p=128))
                wk_sb = [wk_bg[:, i, :] for i in range(ND)]
                bk_sb = wpool.tile([128, NJ], f32, tag="bk")
                nc.sync.dma_start(out=bk_sb, in_=bk_d[:, :])
            else:
                wv_bg = wpool.tile([128, ND, DG], mdt, tag="wvb", name="wv_bg")
                nc.sync.dma_start(
                    out=wv_bg,
                    in_=wv_d[:, :].rearrange("(i p) n -> p i n", p=128))
                wv_sb = [wv_bg[:, i, :] for i in range(ND)]
                bv_sb = wpool.tile([128, DG], f32, tag="bv")
                bv_ap = bv_d[:]
                bv_bcast = bass.AP(
                    tensor=bv_ap.tensor, offset=bv_ap.offset,
                    ap=[[0, 128]] + list(bv_ap.ap))
                nc.sync.dma_start(out=bv_sb, in_=bv_bcast)

        wo_sb = None

        def setup_wo():
            nonlocal wo_sb
            wo_bg = wpool.tile([128, NJ, D], mdt, tag="wob", name="wo_bg")
            nc.sync.dma_start(
                out=wo_bg, in_=wo_d[:, :].rearrange("(j p) n -> p j n", p=128))
            wo_sb = [wo_bg[:, j, :] for j in range(NJ)]

        def load_x(xd, t):
            ts = slice(512 * t, 512 * (t + 1))
            xt_bg = xpool.tile([128, ND, 512], mdt, tag="xt",
                               name="xt_bg", bufs=6)
            nc.sync.dma_start(
                out=xt_bg,
                in_=xd[:, ts].rearrange("(i p) n -> p i n", p=128))
            return [xt_bg[:, i, :] for i in range(ND)]

        def qk_chain(w_sb, b_sb, dest, j, xts, t):
            ts = slice(512 * t, 512 * (t + 1))
            ps = mmp.tile([128, 512], f32, tag="mm")
            for i in range(ND):
                nc.tensor.matmul(
                    out=ps, lhsT=w_sb[i][:, 128 * j:128 * (j + 1)],
                    rhs=xts[i], start=(i == 0), stop=(i == ND - 1))
            nc.vector.tensor_scalar_add(
                out=dest[j][:, ts], in0=ps, scalar1=b_sb[:, j:j + 1])

        def v_chain(xts, s, t):
            ps = mmp.tile([128, 512], f32, tag="mm")
            for i in range(ND):
                nc.tensor.matmul(
                    out=ps, lhsT=xts[i][:, 128 * s:128 * (s + 1)],
                    rhs=wv_sb[i], start=(i == 0), stop=(i == ND - 1))
            kt = 4 * t + s
            v3 = vaug[kt].rearrange("p (h d) -> p h d", h=HG)
            nc.vector.tensor_add(
                v3[:, :, 0:64],
                ps.rearrange("p (h d) -> p h d", h=HG),
                bv_sb.rearrange("p (h d) -> p h d", h=HG))

        def emit_attn(t, mpool):
            qs = slice(512 * t, 512 * (t + 1))
            msk = []
            for hkt in range(4):
                msk_bg = mpool.tile([128, NKT // 4, 512], f32, tag="msk",
                                    name="msk_bg", bufs=6)
                rs = slice(512 * hkt, 512 * (hkt + 1))
                nc.sync.dma_start(
                    out=msk_bg,
                    in_=msk_d[rs, qs].rearrange("(k p) n -> p k n", p=128))
                msk.extend(msk_bg[:, kt, :] for kt in range(NKT // 4))
            ctxt = [ctpool.tile([128, 512], mdt, tag="ct", name="ctxt")
                    for _ in range(NJ)]
            for hp in range(NJ):
                jt = hp
                ctx_ab = [ctxp.tile([128, 512], f32, tag="ctx",
                                    name="ctx_ab") for _ in range(2)]
                for kt in range(NKT):
                    ks = slice(128 * kt, 128 * (kt + 1))
                    ps2 = mm2p.tile([128, 1024], f32, tag="mm2")
                    ps3 = ps2.rearrange("p (h n) -> p h n", h=2)
                    for half in range(2):
                        ro = 64 * half
                        nc.tensor.matmul(
                            out=ps3[:, half, :],
                            lhsT=kt_sb[jt][ro:ro + 64, ks],
                            rhs=qt_sb[jt][ro:ro + 64, qs],
                            start=True, stop=True, skip_group_check=True)
                    pe = ppool.tile([128, 1024], mdt, tag="pexp")
                    pe3 = pe.rearrange("p (h n) -> p h n", h=2)
                    for half in range(2):
                        nc.vector.tensor_add(
                            ps3[:, half, :], ps3[:, half, :], msk[kt])
                    nc.scalar.activation(
                        out=pe3, in_=ps3, func=AF.Exp, bias=0.0)
                    for half in range(2):
                        h = 2 * hp + half
                        nc.tensor.matmul(
                            out=ctx_ab[half][0:65, :],
                            lhsT=vaug[kt][:, 65 * h:65 * (h + 1)],
                            rhs=pe3[:, half, :],
                            start=(kt == 0), stop=(kt == NKT - 1),
                            skip_group_check=True)
                for half in range(2):
                    ro = 64 * half
                    cab = ctx_ab[half]
                    rz = rzpool.tile([1, 512], f32, tag="rz")
                    with nc.allow_low_precision(reason="1/Z bcast operand"):
                        nc.vector.reciprocal(out=rz, in_=cab[64:65, :])
                    rb = rbpool.tile([64, 512], f32, tag="rb")
                    nc.gpsimd.partition_broadcast(
                        rb[:, :], rz[:, :], channels=64)
                    nc.vector.scalar_tensor_tensor(
                        out=ctxt[jt][ro:ro + 64, :],
                        in0=cab[0:64, :],
                        scalar=1.0, in1=rb,
                        op0=ALU.mult, op1=ALU.mult)
            return ctxt

        def out_proj(t, ctxt):
            for s in range(4):
                for e in range(2):
                    es = slice(512 * e, 512 * (e + 1))
                    ps = mmp.tile([128, 512], f32, tag="mm")
                    for jt in range(NJ):
                        nc.tensor.matmul(
                            out=ps,
                            lhsT=ctxt[jt][:, 128 * s:128 * (s + 1)],
                            rhs=wo_sb[jt][:, es],
                            start=(jt == 0), stop=(jt == NJ - 1))
                    ob = opool.tile([128, 512], f32, tag="ob")
                    nc.vector.tensor_copy(out=ob, in_=ps)
                    r0 = 512 * t + 128 * s
                    nc.sync.dma_start(out=out_d[r0:r0 + 128, es], in_=ob)

        with tc.tile_pool(name="xin", bufs=1) as xpool:
            load_w3("q")
            load_w3("k")
            load_w3("v")
            setup_wo()
            for t in range(NT):
                xq = load_x(xqt, t)
                xk = load_x(xkt, t)
                xv = load_x(xvt, t)
                for j in range(NJ):
                    qk_chain(wq_sb, bq_sb, qt_sb, j, xq, t)
                for j in range(NJ):
                    qk_chain(wk_sb, bk_sb, kt_sb, j, xk, t)
                for s in range(4):
                    v_chain(xv, s, t)
        mpool = ep(tc.tile_pool(name="msk", bufs=1))
        for t in range(NT):
            ctxt = emit_attn(t, mpool)
            out_proj(t, ctxt)

    nc.finalize()
    return nc


def _build(mm_dtype_name, causal):
    """Compat wrapper (span.py uses this)."""
    return _build_fast() if causal else _build_generic(mm_dtype_name)


def _get_nc(causal):
    key = ("fast" if causal else MM_DTYPE, causal)
    if key not in _cache:
        _cache[key] = _build_fast() if causal else _build_generic(MM_DTYPE)
    return _cache[key]


last_result = None


def _is_causal(attn_mask):
    tri = np.tril(np.ones((L, L), bool))
    expect = np.where(tri, np.float32(0.0), np.float32(-1e9))
    return np.array_equal(attn_mask, expect)


def _f8split(a):
    hi = a.astype(float8_e4m3)
    lo = (a - hi.astype(np.float32)).astype(float8_e4m3)
    return hi, lo


def kernel(**inputs):
    global last_result
    from concourse.bass_utils import run_bass_kernel_spmd

    inp = {k: np.asarray(v) for k, v in inputs.items()}
    padd = inp["padd_mask"].astype(np.float32)
    am = inp["attn_mask"].astype(np.float32)
    causal = _is_causal(am)

    import os
    trace = bool(os.environ.get("KBENCH_TRACE"))

    def make_generic_maps():
        maskT = np.ascontiguousarray(am.T)
        scale = 1.0 / np.sqrt(np.float32(DH))
        wq_s = (inp["Wq"].astype(np.float32) * scale).astype(bfloat16)
        bq_s = (inp["bq"].astype(np.float32) * scale).astype(np.float32)
        maps = []
        for b in range(B):
            xq = inp["encodings_for_q"][b].astype(
                np.float32).T.astype(bfloat16)
            xk = inp["encodings_for_k"][b].astype(
                np.float32).T.astype(bfloat16)
            xv = inp["encodings_for_v"][b].astype(
                np.float32).T.astype(bfloat16)
            mt = (maskT + padd[b][:, None]).astype(np.float32)
            for g in range(G):
                gs = slice(DG * g, DG * (g + 1))
                maps.append({
                    "xqt": xq, "xkt": xk, "xvt": xv,
                    "wq": np.ascontiguousarray(wq_s[:, gs]),
                    "wk": np.ascontiguousarray(
                        inp["Wk"].astype(np.float32)[:, gs].astype(bfloat16)),
                    "wv": np.ascontiguousarray(
                        inp["Wv"].astype(np.float32)[:, gs].astype(bfloat16)),
                    "bq": np.ascontiguousarray(
                        bq_s[gs].reshape(NJ, 128).T),
                    "bk": np.ascontiguousarray(
                        inp["bk"].astype(np.float32)[gs].reshape(NJ, 128).T),
                    "bv": np.ascontiguousarray(
                        inp["bv"].astype(np.float32)[gs]),
                    "wo": np.ascontiguousarray(
                        inp["Wo"].astype(np.float32)[gs, :].astype(bfloat16)),
                    "maskt": mt,
                })
        return maps

    if causal:
        ident = np.eye(128, dtype=bfloat16)
        in_maps = []
        for b in range(B):
            xmaps = {}
            for nm, key in (("xq", "encodings_for_q"),
                            ("xk", "encodings_for_k"),
                            ("xv", "encodings_for_v")):
                xt = np.ascontiguousarray(inp[key][b].astype(np.float32).T)
                hi, lo = _f8split(xt)
                xmaps[nm + "hi"] = hi
                xmaps[nm + "lo"] = lo
            paddk = np.ascontiguousarray(padd[b].reshape(NKT, 128).T)
            for g in range(G):
                gs = slice(DG * g, DG * (g + 1))
                m = dict(xmaps)
                for nm, W in (("wq", "Wq"), ("wk", "Wk"), ("wv", "Wv")):
                    ws = np.ascontiguousarray(
                        inp[W].astype(np.float32)[:, gs]) * WSCALE
                    hi, lo = _f8split(ws)
                    m[nm + "hi"] = hi
                    if nm == "wv":      # only V uses the W_lo term
                        m[nm + "lo"] = lo
                m["bq"] = np.ascontiguousarray(
                    (inp["bq"].astype(np.float32)[gs] * WSCALE
                     ).reshape(NJ, 128).T)
                m["bk"] = np.ascontiguousarray(
                    (inp["bk"].astype(np.float32)[gs] * WSCALE
                     ).reshape(NJ, 128).T)
                m["bv"] = np.ascontiguousarray(
                    inp["bv"].astype(np.float32)[gs])
                m["wo"] = np.ascontiguousarray(
                    inp["Wo"].astype(np.float32)[gs, :].astype(bfloat16))
                m["paddk"] = paddk
                m["ident"] = ident
                in_maps.append(m)
        try:
            nc = _get_nc(True)
            res = run_bass_kernel_spmd(nc, in_maps, list(range(NCORES)),
                                       trace=trace)
        except Exception:
            nc = _get_nc(False)
            res = run_bass_kernel_spmd(nc, make_generic_maps(),
                                       list(range(NCORES)), trace=trace)
    else:
        nc = _get_nc(False)
        res = run_bass_kernel_spmd(nc, make_generic_maps(),
                                   list(range(NCORES)), trace=trace)

    last_result = res
    out = np.empty((B, L, D), np.float32)
    for b in range(B):
        out[b] = (res.results[2 * b]["out"].astype(np.float32)
                  + res.results[2 * b + 1]["out"].astype(np.float32))
    return out
